# revision 1
# baseline (speedup 1.0000x reference)
"""ChirpTokenizer Trainium2 kernel.

Math: the reference pipeline (hann window -> per-chirp-rate warp resample
with linear interpolation + jacobian -> rFFT over the warped axis) is
linear in x for each chirp rate d.  It therefore collapses into a single
matmul per d:

    out[b, w, d, f] = sum_k x[b, 512*w + k] * G_d[k, f]

where G_d = diag(hann) @ A_d @ F, A_d is the (K x K_TAU) sparse
interpolation/jacobian matrix (2 nnz per column) and F the rFFT matrix.
Since the input is real, Im(X[0]) = Im(X[1024]) = 0, so the packed real
G_d is (1024 x 2048): [Re f=0..1024 | Im f=1..1023].

G_d depends only on dlnf (16 floats); it is built on the host with a
sparse scatter + FFT (cheap) and shipped to the device.  The device does
pure TensorE work: for each core, 2 chirp rates x (2048 rows x 1024 k x
2048 f) fp16 matmuls, PSUM-accumulated over k (fp16 keeps full rel err
~3.5e-4 since products accumulate in fp32 PSUM; it streams faster than
fp32/fp32r through the PE and halves all transfers).

Sharding: D=16 chirp rates over 8 cores (2 per core); frames replicated.
"""

import numpy as np

K = 1024
HOP = 512
K_TAU = 2048
FK = K_TAU // 2 + 1  # 1025
B = 4
N = 262144
D = 16
NWIN = (N - K) // HOP + 1  # 511
NCORES = 8
DPC = D // NCORES  # 2 chirp rates per core
WPAD = 512  # pad 511 windows -> 512 per batch element
ROWS_PAD = B * WPAD  # 2048

_NC_CACHE = {}


def _warp_grid_impl(dlnf):
    """Replicate the reference's f32 warp-grid computation bit-for-bit.

    Runs via jax on CPU: the grid has a 1/beta cancellation that amplifies
    1-ulp exp/log1p differences into ~1e-3-sample index shifts, so the
    exact XLA-CPU op implementations matter.
    """
    import jax.numpy as jnp

    beta = 2.0 * dlnf
    tau = 2.0 * jnp.arange(K_TAU, dtype=jnp.float32) / K_TAU - 1.0
    small = jnp.abs(beta) < 1e-8
    beta_safe = jnp.where(small, 1e-8, beta)
    e2b = jnp.exp(2.0 * beta_safe)

    t_source = (
        jnp.log1p((tau[None, :] + 1.0) * 0.5 * (e2b[:, None] - 1.0))
        / beta_safe[:, None]
        - 1.0
    )
    t_source = jnp.where(small[:, None], tau[None, :], t_source)

    tau_mid = 2.0 * (K_TAU // 2) / K_TAU - 1.0  # = 0.0
    t_mid = jnp.log1p((tau_mid + 1.0) * 0.5 * (e2b - 1.0)) / beta_safe - 1.0
    t_mid = jnp.where(small, tau_mid, t_mid)

    jac = jnp.exp(-beta_safe[:, None] * (t_source - t_mid[:, None]))
    jac = jnp.where(small[:, None], 1.0, jac)

    idx = (K / 2.0) * (t_source + 1.0)
    idx_lo = jnp.clip(jnp.floor(idx).astype(jnp.int32), 0, K - 2)
    frac = idx - idx_lo.astype(jnp.float32)
    return idx_lo, frac, jac


def _warp_grid_np(dlnf):
    """Numpy fallback (used only if no jax CPU backend is available)."""
    f32 = np.float32
    beta = (f32(2.0) * dlnf).astype(f32)
    tau = (f32(2.0) * np.arange(K_TAU, dtype=f32) / f32(K_TAU) - f32(1.0)).astype(f32)
    small = np.abs(beta) < f32(1e-8)
    beta_safe = np.where(small, f32(1e-8), beta).astype(f32)
    e2b = np.exp((f32(2.0) * beta_safe).astype(f32)).astype(f32)
    t_source = (
        np.log1p((tau[None, :] + f32(1.0)) * f32(0.5) * (e2b[:, None] - f32(1.0)))
        / beta_safe[:, None]
        - f32(1.0)
    ).astype(f32)
    t_source = np.where(small[:, None], tau[None, :], t_source).astype(f32)
    t_mid = (
        np.log1p(f32(0.5) * (e2b - f32(1.0))) / beta_safe - f32(1.0)
    ).astype(f32)
    t_mid = np.where(small, f32(0.0), t_mid).astype(f32)
    jac = np.exp(-beta_safe[:, None] * (t_source - t_mid[:, None])).astype(f32)
    jac = np.where(small[:, None], f32(1.0), jac)
    idx = (f32(K / 2.0) * (t_source + f32(1.0))).astype(f32)
    idx_lo = np.clip(np.floor(idx).astype(np.int32), 0, K - 2)
    frac = (idx - idx_lo.astype(f32)).astype(f32)
    return idx_lo, frac, jac


def _warp_grid(dlnf: np.ndarray):
    dlnf = np.asarray(dlnf, np.float32)
    try:
        import jax

        if "warp_jit" not in _NC_CACHE:
            cpu = jax.local_devices(backend="cpu")[0]
            _NC_CACHE["warp_jit"] = jax.jit(_warp_grid_impl, device=cpu)
        idx_lo, frac, jac = _NC_CACHE["warp_jit"](dlnf)
        return np.asarray(idx_lo), np.asarray(frac), np.asarray(jac)
    except Exception:
        return _warp_grid_np(dlnf)


def _build_g(dlnf: np.ndarray) -> np.ndarray:
    """(D,) f32 -> (D, 8, 128, 2048) fp16: packed DFT-of-resample matrices."""
    f32 = np.float32
    dlnf = np.asarray(dlnf, f32)
    idx_lo, frac, jac = _warp_grid(dlnf)

    # fold the hann window (a function of the source row k) into the
    # interpolation weights so no extra pass over G is needed
    n = np.arange(K, dtype=np.float64)
    hann = (0.5 - 0.5 * np.cos(2.0 * np.pi * n / K)).astype(f32)
    lo = idx_lo.ravel()
    w_lo = ((f32(1.0) - frac) * jac).ravel() * hann[lo]
    w_hi = (frac * jac).ravel() * hann[lo + 1]

    # A[d, k, t]: sparse scatter (indices are unique — lo vs lo+1 never
    # collide for the same t, and t differs otherwise)
    A = np.zeros((D, K, K_TAU), f32)
    d_idx = np.repeat(np.arange(D), K_TAU)
    t_idx = np.tile(np.arange(K_TAU), D)
    A[d_idx, lo, t_idx] = w_lo
    A[d_idx, lo + 1, t_idx] = w_hi

    try:
        from scipy.fft import rfft as _rfft

        W = _rfft(A, axis=-1, workers=-1)  # (D, K, FK) complex64
    except ImportError:
        W = np.fft.rfft(A, axis=-1)

    G = np.empty((D, K, 2048), np.float16)
    G[:, :, :FK] = W.real
    G[:, :, FK:] = W.imag[:, :, 1:1024]
    return np.ascontiguousarray(G.reshape(D, 8, 128, 2048))


def _build_frames_t(x: np.ndarray) -> np.ndarray:
    """(B, N) f32 -> (8, 128, ROWS_PAD) fp16 transposed overlapped frames.

    ft[kc, i, b*512 + w] = x[b, 512*w + 128*kc + i]  (w < 511; w = 511 zero)
    """
    ft = np.zeros((K, ROWS_PAD), np.float16)
    for b in range(B):
        frames = np.lib.stride_tricks.as_strided(
            x[b], shape=(NWIN, K), strides=(HOP * 4, 4)
        )
        ft[:, b * WPAD : b * WPAD + NWIN] = frames.T.astype(np.float16)
    return np.ascontiguousarray(ft.reshape(8, 128, ROWS_PAD))


def _get_nc():
    if "nc" in _NC_CACHE:
        return _NC_CACHE["nc"]
    import concourse.bacc as bacc
    import concourse.mybir as mybir
    from concourse import tile

    nc = bacc.Bacc("TRN2", target_bir_lowering=False, debug=False, num_devices=NCORES)
    ft_d = nc.dram_tensor(
        "ft", [8, 128, ROWS_PAD], mybir.dt.float16, kind="ExternalInput"
    )
    g_d = nc.dram_tensor(
        "g", [DPC, 8, 128, 2048], mybir.dt.float16, kind="ExternalInput"
    )
    out_d = nc.dram_tensor(
        "out", [DPC, ROWS_PAD, 2048], mybir.dt.float16, kind="ExternalOutput"
    )

    with tile.TileContext(nc) as tc:
        with (
            tc.tile_pool(name="ftp", bufs=8) as ftp,
            tc.tile_pool(name="gp", bufs=16) as gp,
            tc.tile_pool(name="op", bufs=4) as op,
            tc.tile_pool(name="pp", bufs=8, space="PSUM") as pp,
        ):
            ftt = []
            for kc in range(8):
                t = ftp.tile(
                    [128, ROWS_PAD], mybir.dt.float16, name=f"ft{kc}", tag="ft"
                )
                nc.sync.dma_start(t[:], ft_d[kc])
                ftt.append(t)
            for d in range(DPC):
                gtt = []
                for kc in range(8):
                    t = gp.tile(
                        [128, 2048], mybir.dt.float16, name=f"g{d}_{kc}", tag="g"
                    )
                    nc.sync.dma_start(t[:], g_d[d, kc])
                    gtt.append(t)
                for m in range(16):
                    ost = op.tile(
                        [128, 2048], mybir.dt.float16, name=f"o{d}_{m}", tag="o"
                    )
                    pss = [
                        pp.tile(
                            [128, 512],
                            mybir.dt.float32,
                            name=f"p{d}_{m}_{nn}",
                            tag="p",
                        )
                        for nn in range(4)
                    ]
                    # k outer / n inner: the first matmuls only need the first
                    # ft/g k-chunks, so compute starts while later chunks are
                    # still in flight (removes the 16MB startup DMA bubble).
                    for kc in range(8):
                        for nn in range(4):
                            nc.tensor.matmul(
                                pss[nn][:],
                                ftt[kc][:, 128 * m : 128 * (m + 1)],
                                gtt[kc][:, 512 * nn : 512 * (nn + 1)],
                                start=(kc == 0),
                                stop=(kc == 7),
                            )
                    for nn in range(4):
                        nc.vector.tensor_copy(
                            ost[:, 512 * nn : 512 * (nn + 1)], pss[nn][:]
                        )
                    nc.sync.dma_start(out_d[d, 128 * m : 128 * (m + 1), :], ost[:])
    nc.compile()
    _NC_CACHE["nc"] = nc
    return nc


def _get_runner():
    """Build (once) a sharded jitted callable over the 8 cores.

    Mirrors the multi-core tail of bass2jax.run_bass_via_pjrt, but caches
    the jitted function so repeat kernel() calls don't re-trace/re-compile.
    Returns (fn, in_names, out_names, out_shapes_dtypes).
    """
    if "runner" in _NC_CACHE:
        return _NC_CACHE["runner"]
    import jax
    import concourse.mybir as mybir
    from concourse import bass2jax
    from jax.sharding import Mesh, PartitionSpec
    from jax.experimental.shard_map import shard_map

    nc = _get_nc()
    bass2jax.install_neuronx_cc_hook()

    partition_name = (
        nc.partition_id_tensor.name if nc.partition_id_tensor is not None else None
    )
    in_names = []
    out_names = []
    out_avals = []
    for alloc in nc.m.functions[0].allocations:
        if not isinstance(alloc, mybir.MemoryLocationSet):
            continue
        name = alloc.memorylocations[0].name
        if alloc.kind == "ExternalInput":
            if name != partition_name:
                in_names.append(name)
        elif alloc.kind == "ExternalOutput":
            shape = tuple(alloc.tensor_shape)
            dtype = mybir.dt.np(alloc.dtype)
            out_names.append(name)
            out_avals.append(jax.core.ShapedArray(shape, dtype))
    n_params = len(in_names)
    n_outs = len(out_names)
    all_names = list(in_names) + list(out_names)
    if partition_name is not None:
        all_names.append(partition_name)
    all_names = tuple(all_names)

    def _body(*args):
        operands = list(args)
        if partition_name is not None:
            operands.append(bass2jax.partition_id_tensor())
        outs = bass2jax._bass_exec_p.bind(
            *operands,
            out_avals=tuple(out_avals),
            in_names=all_names,
            out_names=tuple(out_names),
            lowering_input_output_aliases=(),
            sim_require_finite=True,
            sim_require_nnan=True,
            nc=nc,
        )
        return tuple(outs)

    devices = jax.devices()[:NCORES]
    mesh = Mesh(np.asarray(devices), ("core",))
    # ft is identical on every core: pass it replicated (one wire transfer)
    # instead of 8x-concatenated; every other operand is sharded on axis 0.
    in_specs = tuple(
        PartitionSpec() if name == "ft" else PartitionSpec("core")
        for name in in_names
    ) + (PartitionSpec("core"),) * n_outs
    out_specs = (PartitionSpec("core"),) * n_outs
    fn = jax.jit(
        shard_map(
            _body, mesh=mesh, in_specs=in_specs, out_specs=out_specs, check_rep=False
        ),
        donate_argnums=tuple(range(n_params, n_params + n_outs)),
        keep_unused=True,
    )
    runner = (fn, in_names, out_names, [(a.shape, a.dtype) for a in out_avals], mesh)
    _NC_CACHE["runner"] = runner
    return runner


def kernel(x: np.ndarray, dlnf: np.ndarray) -> np.ndarray:
    x = np.ascontiguousarray(np.asarray(x, dtype=np.float32))
    dlnf = np.asarray(dlnf, dtype=np.float32)

    G = _build_g(dlnf)  # (D, 8, 128, 2048)
    FT = _build_frames_t(x)  # (8, 128, ROWS_PAD)

    fn, in_names, out_names, out_sd, _mesh = _get_runner()
    full_in = {"ft": FT, "g": np.ascontiguousarray(G.reshape(NCORES * DPC, 8, 128, 2048))}
    concat_in = [full_in[name] for name in in_names]
    concat_zeros = [
        np.zeros((NCORES * s[0], *s[1:]), dt) for (s, dt) in out_sd
    ]
    out_arrs = fn(*concat_in, *concat_zeros)
    o_all = np.asarray(out_arrs[out_names.index("out")]).reshape(
        NCORES, DPC, ROWS_PAD, 2048
    )

    # o5[d, b, w, :] with w < NWIN valid; assemble into interleaved complex64
    # via a float32 view so the fp16->f32 cast fuses into the strided copies
    o5 = o_all.reshape(D, B, WPAD, 2048)[:, :, :NWIN, :]
    out_f = np.empty((B, NWIN, D, FK, 2), np.float32)
    out_f[..., 0] = np.transpose(o5[:, :, :, :FK], (1, 2, 0, 3))
    out_f[:, :, :, 0, 1] = 0.0
    out_f[:, :, :, 1024, 1] = 0.0
    out_f[:, :, :, 1:1024, 1] = np.transpose(o5[:, :, :, FK:], (1, 2, 0, 3))
    return out_f.view(np.complex64)[..., 0]



# revision 29
# speedup vs baseline: 1.9955x; 1.9955x over previous
"""ChirpTokenizer Trainium2 kernel (fp8 DoubleRow edition).

Math: the reference pipeline (hann window -> per-chirp-rate warp resample
with linear interpolation + jacobian -> rFFT over the warped axis) is
linear in x for each chirp rate d.  It therefore collapses into a single
matmul per d:

    out[b, w, d, f] = sum_k x[b, 512*w + k] * G_d[k, f]

where G_d = diag(hann) @ A_d @ F, A_d is the (K x K_TAU) sparse
interpolation/jacobian matrix (2 nnz per column) and F the rFFT matrix.
Since the input is real, Im(X[0]) = Im(X[1024]) = 0, so the packed real
G_d is (1024 x 2048): [Re f=0..1024 | Im f=1..1023].

Device compute: fp8(e4m3) matmuls in MatmulPerfMode.DoubleRow, which
contracts TWO 128-deep k-chunks per instruction at 0.5 cycles/row --
4x the fp16 rate.  Raw fp8 quantization of both operands costs ~3.7%
relative error (gate is 2e-2), so we use a first-order residual split:

    x @ G ~= x8 @ G8  +  xr8 @ G8  +  x8 @ Gr8

with x8 = q8(x), xr8 = q8(x - x8), G8 = q8(G), Gr8 = q8(G - G8); the
dropped xr*Gr term and the residual requantization leave ~1.5e-3 rel
error (measured).  Three fp8-DR passes cost 0.75x one fp16 pass.

Sharding: D=16 chirp rates over 8 cores (2 per core); frames replicated.
"""

import numpy as np
import ml_dtypes

K = 1024
HOP = 512
K_TAU = 2048
FK = K_TAU // 2 + 1  # 1025
B = 4
N = 262144
D = 16
NWIN = (N - K) // HOP + 1  # 511
NCORES = 8
DPC = D // NCORES  # 2 chirp rates per core
WPAD = 512  # pad 511 windows -> 512 per batch element
ROWS_PAD = B * WPAD  # 2048
NJ = 4  # k-chunk pairs: K = NJ * 2 * 128
INNER_J = (1, 2)  # k-chunk pairs that get residual correction (high hann)

# numpy dtype matching mybir.dt.float8e4 (== mybir.dt.np(mybir.dt.float8e4))
F8 = ml_dtypes.float8_e4m3

_NC_CACHE = {}


def _warp_grid_impl(dlnf):
    """Replicate the reference's f32 warp-grid computation bit-for-bit.

    Runs via jax on CPU: the grid has a 1/beta cancellation that amplifies
    1-ulp exp/log1p differences into ~1e-3-sample index shifts, so the
    exact XLA-CPU op implementations matter.
    """
    import jax.numpy as jnp

    beta = 2.0 * dlnf
    tau = 2.0 * jnp.arange(K_TAU, dtype=jnp.float32) / K_TAU - 1.0
    small = jnp.abs(beta) < 1e-8
    beta_safe = jnp.where(small, 1e-8, beta)
    e2b = jnp.exp(2.0 * beta_safe)

    t_source = (
        jnp.log1p((tau[None, :] + 1.0) * 0.5 * (e2b[:, None] - 1.0))
        / beta_safe[:, None]
        - 1.0
    )
    t_source = jnp.where(small[:, None], tau[None, :], t_source)

    tau_mid = 2.0 * (K_TAU // 2) / K_TAU - 1.0  # = 0.0
    t_mid = jnp.log1p((tau_mid + 1.0) * 0.5 * (e2b - 1.0)) / beta_safe - 1.0
    t_mid = jnp.where(small, tau_mid, t_mid)

    jac = jnp.exp(-beta_safe[:, None] * (t_source - t_mid[:, None]))
    jac = jnp.where(small[:, None], 1.0, jac)

    idx = (K / 2.0) * (t_source + 1.0)
    idx_lo = jnp.clip(jnp.floor(idx).astype(jnp.int32), 0, K - 2)
    frac = idx - idx_lo.astype(jnp.float32)
    return idx_lo, frac, jac


def _warp_grid_np(dlnf):
    """Numpy fallback (used only if no jax CPU backend is available)."""
    f32 = np.float32
    beta = (f32(2.0) * dlnf).astype(f32)
    tau = (f32(2.0) * np.arange(K_TAU, dtype=f32) / f32(K_TAU) - f32(1.0)).astype(f32)
    small = np.abs(beta) < f32(1e-8)
    beta_safe = np.where(small, f32(1e-8), beta).astype(f32)
    e2b = np.exp((f32(2.0) * beta_safe).astype(f32)).astype(f32)
    t_source = (
        np.log1p((tau[None, :] + f32(1.0)) * f32(0.5) * (e2b[:, None] - f32(1.0)))
        / beta_safe[:, None]
        - f32(1.0)
    ).astype(f32)
    t_source = np.where(small[:, None], tau[None, :], t_source).astype(f32)
    t_mid = (
        np.log1p(f32(0.5) * (e2b - f32(1.0))) / beta_safe - f32(1.0)
    ).astype(f32)
    t_mid = np.where(small, f32(0.0), t_mid).astype(f32)
    jac = np.exp(-beta_safe[:, None] * (t_source - t_mid[:, None])).astype(f32)
    jac = np.where(small[:, None], f32(1.0), jac)
    idx = (f32(K / 2.0) * (t_source + f32(1.0))).astype(f32)
    idx_lo = np.clip(np.floor(idx).astype(np.int32), 0, K - 2)
    frac = (idx - idx_lo.astype(f32)).astype(f32)
    return idx_lo, frac, jac


def _warp_grid(dlnf: np.ndarray):
    dlnf = np.asarray(dlnf, np.float32)
    try:
        import jax

        if "warp_jit" not in _NC_CACHE:
            cpu = jax.local_devices(backend="cpu")[0]
            _NC_CACHE["warp_jit"] = jax.jit(_warp_grid_impl, device=cpu)
        idx_lo, frac, jac = _NC_CACHE["warp_jit"](dlnf)
        return np.asarray(idx_lo), np.asarray(frac), np.asarray(jac)
    except Exception:
        return _warp_grid_np(dlnf)


def _build_g32(dlnf: np.ndarray) -> np.ndarray:
    """(D,) f32 -> (D, 1024, 2048) f32: packed DFT-of-resample matrices."""
    f32 = np.float32
    dlnf = np.asarray(dlnf, f32)
    idx_lo, frac, jac = _warp_grid(dlnf)

    # fold the hann window (a function of the source row k) into the
    # interpolation weights so no extra pass over G is needed
    n = np.arange(K, dtype=np.float64)
    hann = (0.5 - 0.5 * np.cos(2.0 * np.pi * n / K)).astype(f32)
    lo = idx_lo.ravel()
    w_lo = ((f32(1.0) - frac) * jac).ravel() * hann[lo]
    w_hi = (frac * jac).ravel() * hann[lo + 1]

    # A[d, k, t]: sparse scatter (indices are unique -- lo vs lo+1 never
    # collide for the same t, and t differs otherwise)
    A = np.zeros((D, K, K_TAU), f32)
    d_idx = np.repeat(np.arange(D), K_TAU)
    t_idx = np.tile(np.arange(K_TAU), D)
    A[d_idx, lo, t_idx] = w_lo
    A[d_idx, lo + 1, t_idx] = w_hi

    try:
        from scipy.fft import rfft as _rfft

        W = _rfft(A, axis=-1, workers=-1)  # (D, K, FK) complex64
    except ImportError:
        W = np.fft.rfft(A, axis=-1)

    G = np.empty((D, K, 2048), f32)
    G[:, :, :FK] = W.real
    G[:, :, FK:] = W.imag[:, :, 1:1024]
    return G


def _build_frames32(x: np.ndarray) -> np.ndarray:
    """(B, N) f32 -> (K, ROWS_PAD) f32 transposed overlapped frames.

    ft[k, b*512 + w] = x[b, 512*w + k]  (w < 511; w = 511 zero)
    """
    ft = np.zeros((K, ROWS_PAD), np.float32)
    for b in range(B):
        frames = np.lib.stride_tricks.as_strided(
            x[b], shape=(NWIN, K), strides=(HOP * 4, 4)
        )
        ft[:, b * WPAD : b * WPAD + NWIN] = frames.T
    return ft


def _split8(a32: np.ndarray):
    """f32 -> (main fp8, residual fp8) with residual = q8(a - f32(main))."""
    a8 = a32.astype(F8)
    r8 = (a32 - a8.astype(np.float32)).astype(F8)
    return a8, r8


def _to_pairs(a: np.ndarray) -> np.ndarray:
    """(..., K, X) -> (..., NJ, 128, 2, X): k = 128*(2j + i) + p -> [j, p, i]."""
    lead = a.shape[:-2]
    x = a.shape[-1]
    a = a.reshape(*lead, NJ, 2, 128, x)
    order = tuple(range(len(lead))) + (
        len(lead),
        len(lead) + 2,
        len(lead) + 1,
        len(lead) + 3,
    )
    return np.ascontiguousarray(np.transpose(a, order))


def _prep_inputs(x: np.ndarray, dlnf: np.ndarray) -> dict:
    """Host prep: full-shape (pre-shard) device input arrays by name."""
    ft32 = _build_frames32(np.ascontiguousarray(np.asarray(x, np.float32)))
    g32 = _build_g32(np.asarray(dlnf, np.float32))
    ft8, ftr8 = _split8(ft32)
    g8, gr8 = _split8(g32)
    return {
        "ft8": _to_pairs(ft8),  # (4, 128, 2, ROWS_PAD) replicated
        "ftr8": _to_pairs(ftr8),  # (4, 128, 2, ROWS_PAD) replicated
        "g8": _to_pairs(g8),  # (D, 4, 128, 2, 2048) sharded on D
        "gr8": _to_pairs(gr8),  # (D, 4, 128, 2, 2048) sharded on D
    }


_REPLICATED = {"ft8", "ftr8"}


def _get_nc():
    if "nc" in _NC_CACHE:
        return _NC_CACHE["nc"]
    import concourse.bacc as bacc
    import concourse.mybir as mybir
    from concourse import tile

    nc = bacc.Bacc("TRN2", target_bir_lowering=False, debug=False, num_devices=NCORES)
    f8 = mybir.dt.float8e4
    ft8_d = nc.dram_tensor("ft8", [NJ, 128, 2, ROWS_PAD], f8, kind="ExternalInput")
    ftr8_d = nc.dram_tensor("ftr8", [NJ, 128, 2, ROWS_PAD], f8, kind="ExternalInput")
    g8_d = nc.dram_tensor("g8", [DPC, NJ, 128, 2, 2048], f8, kind="ExternalInput")
    gr8_d = nc.dram_tensor("gr8", [DPC, NJ, 128, 2, 2048], f8, kind="ExternalInput")
    out_d = nc.dram_tensor(
        "out", [DPC, ROWS_PAD, 2048], mybir.dt.float16, kind="ExternalOutput"
    )
    DR = mybir.MatmulPerfMode.DoubleRow

    with tile.TileContext(nc) as tc:
        with (
            tc.tile_pool(name="ftp", bufs=32) as ftp,
            tc.tile_pool(name="gp", bufs=24) as gp,
            tc.tile_pool(name="op", bufs=6) as op,
            tc.tile_pool(name="pp", bufs=8, space="PSUM") as pp,
        ):
            # DMA granularity is chosen so the first PSUM group's 16-tile
            # working set closes as early as possible: G comes in halves
            # (790ns each), frames in row-quads (500ns floor each), issued
            # interleaved in first-use order.
            def dma_g_j(d, j, g_t, gr_t, which=("g", "gr"), halves=(0, 1)):
                for name, dram, lst in (
                    ("g", g8_d, g_t),
                    ("gr", gr8_d, gr_t),
                ):
                    if name not in which:
                        continue
                    for h in halves:
                        t = gp.tile(
                            [128, 2, 1024], f8, name=f"{name}{d}_{j}_{h}", tag=name
                        )
                        nc.sync.dma_start(
                            t[:], dram[d, j, :, :, 1024 * h : 1024 * (h + 1)]
                        )
                        lst[j][h] = t

            # frames: quads of 512 rows -> ft_t[j][q], ftr_t[j][q]
            ft_t = [[None] * 4 for _ in range(NJ)]
            ftr_t = [[None] * 4 for _ in range(NJ)]

            def dma_ft(j, q):
                t = ftp.tile([128, 2, 512], f8, name=f"ft{j}_{q}", tag="ft")
                nc.sync.dma_start(t[:], ft8_d[j, :, :, 512 * q : 512 * (q + 1)])
                ft_t[j][q] = t
                if j in INNER_J:
                    t = ftp.tile([128, 2, 512], f8, name=f"ftr{j}_{q}", tag="ftr")
                    nc.sync.dma_start(t[:], ftr8_d[j, :, :, 512 * q : 512 * (q + 1)])
                    ftr_t[j][q] = t

            # per-j issue order matches per-j consumption order of the first
            # PSUM half-group (h=0: g h0, ft q0, ftr q0, gr h0); h1 follows
            # issue ALL input DMAs upfront (SP queue), in first-use order;
            # output DMAs go on the Activation HWDGE queue so they cannot
            # head-of-line block the input feed
            gs = [
                [[None, None] for _ in range(NJ)],
                [[None, None] for _ in range(NJ)],
            ]
            grs = [
                [[None, None] for _ in range(NJ)],
                [[None, None] for _ in range(NJ)],
            ]
            for j in range(NJ):
                dma_g_j(0, j, gs[0], grs[0], which=("g",), halves=(0,))
                dma_ft(j, 0)
                if j in INNER_J:
                    dma_g_j(0, j, gs[0], grs[0], which=("gr",), halves=(0,))
            for q in range(1, 4):
                for j in range(NJ):
                    dma_ft(j, q)
            for j in range(NJ):
                dma_g_j(
                    0, j, gs[0], grs[0],
                    which=("g", "gr") if j in INNER_J else ("g",),
                    halves=(1,),
                )
            for d in range(1, DPC):
                for h in range(2):
                    for j in range(NJ):
                        dma_g_j(
                            d, j, gs[d], grs[d],
                            which=("g", "gr") if j in INNER_J else ("g",),
                            halves=(h,),
                        )

            for d in range(DPC):
                g_t, gr_t = gs[d], grs[d]
                for h in range(2):
                    for m in range(16):
                        q, lm = divmod(m, 4)
                        gi = (2 * d + h) * 16 + m  # global half-group index
                        ost = op.tile(
                            [128, 1024], mybir.dt.float16, name=f"o{d}_{m}_{h}", tag="o"
                        )
                        pss = [
                            pp.tile(
                                [128, 512],
                                mybir.dt.float32,
                                name=f"p{d}_{m}_{h}_{nn}",
                                tag="p",
                            )
                            for nn in range(2)
                        ]
                        # edge k-chunk pairs (j=0,3) carry ~8% of the output
                        # energy (hann-weighted rows), so they get only the
                        # main fp8 term; inner pairs get the full 3-way split
                        steps = []
                        for j in range(NJ):
                            nsi = 3 if j in INNER_J else 1
                            steps.extend((j, si) for si in range(nsi))
                        for stepi, (j, si) in enumerate(steps):
                            lt = ftr_t[j][q] if si == 1 else ft_t[j][q]
                            rt = gr_t[j][h] if si == 2 else g_t[j][h]
                            for nn in range(2):
                                nc.tensor.matmul(
                                    pss[nn][:],
                                    lt[:, :, 128 * lm : 128 * (lm + 1)],
                                    rt[:, :, 512 * nn : 512 * (nn + 1)],
                                    start=(stepi == 0),
                                    stop=(stepi == len(steps) - 1),
                                    perf_mode=DR,
                                )
                        # evictions on DVE; out-DMAs issue on Act early (SP
                        # is still issuing inputs), on SP later
                        nc.vector.tensor_copy(ost[:, 0:512], pss[0][:])
                        nc.vector.tensor_copy(ost[:, 512:1024], pss[1][:])
                        out_eng = nc.scalar if gi < 20 else nc.sync
                        out_eng.dma_start(
                            out_d[
                                d,
                                128 * m : 128 * (m + 1),
                                1024 * h : 1024 * (h + 1),
                            ],
                            ost[:],
                        )
    nc.compile()
    _NC_CACHE["nc"] = nc
    return nc


def _get_runner():
    """Build (once) a sharded jitted callable over the 8 cores.

    Mirrors the multi-core tail of bass2jax.run_bass_via_pjrt, but caches
    the jitted function so repeat kernel() calls don't re-trace/re-compile.
    Returns (fn, in_names, out_names, out_shapes_dtypes, mesh).
    """
    if "runner" in _NC_CACHE:
        return _NC_CACHE["runner"]
    import jax
    import concourse.mybir as mybir
    from concourse import bass2jax
    from jax.sharding import Mesh, PartitionSpec
    from jax.experimental.shard_map import shard_map

    nc = _get_nc()
    bass2jax.install_neuronx_cc_hook()

    partition_name = (
        nc.partition_id_tensor.name if nc.partition_id_tensor is not None else None
    )
    in_names = []
    out_names = []
    out_avals = []
    for alloc in nc.m.functions[0].allocations:
        if not isinstance(alloc, mybir.MemoryLocationSet):
            continue
        name = alloc.memorylocations[0].name
        if alloc.kind == "ExternalInput":
            if name != partition_name:
                in_names.append(name)
        elif alloc.kind == "ExternalOutput":
            shape = tuple(alloc.tensor_shape)
            dtype = mybir.dt.np(alloc.dtype)
            out_names.append(name)
            out_avals.append(jax.core.ShapedArray(shape, dtype))
    n_params = len(in_names)
    n_outs = len(out_names)
    all_names = list(in_names) + list(out_names)
    if partition_name is not None:
        all_names.append(partition_name)
    all_names = tuple(all_names)

    def _body(*args):
        operands = list(args)
        if partition_name is not None:
            operands.append(bass2jax.partition_id_tensor())
        outs = bass2jax._bass_exec_p.bind(
            *operands,
            out_avals=tuple(out_avals),
            in_names=all_names,
            out_names=tuple(out_names),
            lowering_input_output_aliases=(),
            sim_require_finite=True,
            sim_require_nnan=True,
            nc=nc,
        )
        return tuple(outs)

    devices = jax.devices()[:NCORES]
    mesh = Mesh(np.asarray(devices), ("core",))
    # frames are identical on every core: pass them replicated (one wire
    # transfer); every other operand is sharded on axis 0.
    in_specs = tuple(
        PartitionSpec() if name in _REPLICATED else PartitionSpec("core")
        for name in in_names
    ) + (PartitionSpec("core"),) * n_outs
    out_specs = (PartitionSpec("core"),) * n_outs
    fn = jax.jit(
        shard_map(
            _body, mesh=mesh, in_specs=in_specs, out_specs=out_specs, check_rep=False
        ),
        donate_argnums=tuple(range(n_params, n_params + n_outs)),
        keep_unused=True,
    )
    runner = (fn, in_names, out_names, [(a.shape, a.dtype) for a in out_avals], mesh)
    _NC_CACHE["runner"] = runner
    return runner


def kernel(x: np.ndarray, dlnf: np.ndarray) -> np.ndarray:
    full_in = _prep_inputs(x, dlnf)

    fn, in_names, out_names, out_sd, _mesh = _get_runner()
    concat_in = [np.ascontiguousarray(full_in[name]) for name in in_names]
    concat_zeros = [np.zeros((NCORES * s[0], *s[1:]), dt) for (s, dt) in out_sd]
    out_arrs = fn(*concat_in, *concat_zeros)
    o_all = np.asarray(out_arrs[out_names.index("out")]).reshape(
        NCORES, DPC, ROWS_PAD, 2048
    )

    # o5[d, b, w, :] with w < NWIN valid; assemble into interleaved complex64
    # via a float32 view so the fp16->f32 cast fuses into the strided copies
    o5 = o_all.reshape(D, B, WPAD, 2048)[:, :, :NWIN, :]
    out_f = np.empty((B, NWIN, D, FK, 2), np.float32)
    out_f[..., 0] = np.transpose(o5[:, :, :, :FK], (1, 2, 0, 3))
    out_f[:, :, :, 0, 1] = 0.0
    out_f[:, :, :, 1024, 1] = 0.0
    out_f[:, :, :, 1:1024, 1] = np.transpose(o5[:, :, :, FK:], (1, 2, 0, 3))
    return out_f.view(np.complex64)[..., 0]


# revision 41
# speedup vs baseline: 1.9980x; 1.0013x over previous
"""ChirpTokenizer Trainium2 kernel (fp8 DoubleRow edition).

Math: the reference pipeline (hann window -> per-chirp-rate warp resample
with linear interpolation + jacobian -> rFFT over the warped axis) is
linear in x for each chirp rate d.  It therefore collapses into a single
matmul per d:

    out[b, w, d, f] = sum_k x[b, 512*w + k] * G_d[k, f]

where G_d = diag(hann) @ A_d @ F, A_d is the (K x K_TAU) sparse
interpolation/jacobian matrix (2 nnz per column) and F the rFFT matrix.
Since the input is real, Im(X[0]) = Im(X[1024]) = 0, so the packed real
G_d is (1024 x 2048): [Re f=0..1024 | Im f=1..1023].

Device compute: fp8(e4m3) matmuls in MatmulPerfMode.DoubleRow, which
contracts TWO 128-deep k-chunks per instruction at 0.5 cycles/row --
4x the fp16 rate.  Raw fp8 quantization of both operands costs ~3.7%
relative error (gate is 2e-2), so we use a first-order residual split:

    x @ G ~= x8 @ G8  +  xr8 @ G8  +  x8 @ Gr8

with x8 = q8(x), xr8 = q8(x - x8), G8 = q8(G), Gr8 = q8(G - G8); the
dropped xr*Gr term and the residual requantization leave ~1.5e-3 rel
error (measured).  Three fp8-DR passes cost 0.75x one fp16 pass.

Sharding: D=16 chirp rates over 8 cores (2 per core); frames replicated.
"""

import numpy as np
import ml_dtypes

K = 1024
HOP = 512
K_TAU = 2048
FK = K_TAU // 2 + 1  # 1025
B = 4
N = 262144
D = 16
NWIN = (N - K) // HOP + 1  # 511
NCORES = 8
DPC = D // NCORES  # 2 chirp rates per core
WPAD = 512  # pad 511 windows -> 512 per batch element
ROWS_PAD = B * WPAD  # 2048
NJ = 4  # k-chunk pairs: K = NJ * 2 * 128
INNER_J = (1, 2)  # k-chunk pairs that get residual correction (high hann)

# numpy dtype matching mybir.dt.float8e4 (== mybir.dt.np(mybir.dt.float8e4))
F8 = ml_dtypes.float8_e4m3

_NC_CACHE = {}


def _warp_grid_impl(dlnf):
    """Replicate the reference's f32 warp-grid computation bit-for-bit.

    Runs via jax on CPU: the grid has a 1/beta cancellation that amplifies
    1-ulp exp/log1p differences into ~1e-3-sample index shifts, so the
    exact XLA-CPU op implementations matter.
    """
    import jax.numpy as jnp

    beta = 2.0 * dlnf
    tau = 2.0 * jnp.arange(K_TAU, dtype=jnp.float32) / K_TAU - 1.0
    small = jnp.abs(beta) < 1e-8
    beta_safe = jnp.where(small, 1e-8, beta)
    e2b = jnp.exp(2.0 * beta_safe)

    t_source = (
        jnp.log1p((tau[None, :] + 1.0) * 0.5 * (e2b[:, None] - 1.0))
        / beta_safe[:, None]
        - 1.0
    )
    t_source = jnp.where(small[:, None], tau[None, :], t_source)

    tau_mid = 2.0 * (K_TAU // 2) / K_TAU - 1.0  # = 0.0
    t_mid = jnp.log1p((tau_mid + 1.0) * 0.5 * (e2b - 1.0)) / beta_safe - 1.0
    t_mid = jnp.where(small, tau_mid, t_mid)

    jac = jnp.exp(-beta_safe[:, None] * (t_source - t_mid[:, None]))
    jac = jnp.where(small[:, None], 1.0, jac)

    idx = (K / 2.0) * (t_source + 1.0)
    idx_lo = jnp.clip(jnp.floor(idx).astype(jnp.int32), 0, K - 2)
    frac = idx - idx_lo.astype(jnp.float32)
    return idx_lo, frac, jac


def _warp_grid_np(dlnf):
    """Numpy fallback (used only if no jax CPU backend is available)."""
    f32 = np.float32
    beta = (f32(2.0) * dlnf).astype(f32)
    tau = (f32(2.0) * np.arange(K_TAU, dtype=f32) / f32(K_TAU) - f32(1.0)).astype(f32)
    small = np.abs(beta) < f32(1e-8)
    beta_safe = np.where(small, f32(1e-8), beta).astype(f32)
    e2b = np.exp((f32(2.0) * beta_safe).astype(f32)).astype(f32)
    t_source = (
        np.log1p((tau[None, :] + f32(1.0)) * f32(0.5) * (e2b[:, None] - f32(1.0)))
        / beta_safe[:, None]
        - f32(1.0)
    ).astype(f32)
    t_source = np.where(small[:, None], tau[None, :], t_source).astype(f32)
    t_mid = (
        np.log1p(f32(0.5) * (e2b - f32(1.0))) / beta_safe - f32(1.0)
    ).astype(f32)
    t_mid = np.where(small, f32(0.0), t_mid).astype(f32)
    jac = np.exp(-beta_safe[:, None] * (t_source - t_mid[:, None])).astype(f32)
    jac = np.where(small[:, None], f32(1.0), jac)
    idx = (f32(K / 2.0) * (t_source + f32(1.0))).astype(f32)
    idx_lo = np.clip(np.floor(idx).astype(np.int32), 0, K - 2)
    frac = (idx - idx_lo.astype(f32)).astype(f32)
    return idx_lo, frac, jac


def _warp_grid(dlnf: np.ndarray):
    dlnf = np.asarray(dlnf, np.float32)
    try:
        import jax

        if "warp_jit" not in _NC_CACHE:
            cpu = jax.local_devices(backend="cpu")[0]
            _NC_CACHE["warp_jit"] = jax.jit(_warp_grid_impl, device=cpu)
        idx_lo, frac, jac = _NC_CACHE["warp_jit"](dlnf)
        return np.asarray(idx_lo), np.asarray(frac), np.asarray(jac)
    except Exception:
        return _warp_grid_np(dlnf)


def _build_g32(dlnf: np.ndarray) -> np.ndarray:
    """(D,) f32 -> (D, 1024, 2048) f32: packed DFT-of-resample matrices."""
    f32 = np.float32
    dlnf = np.asarray(dlnf, f32)
    idx_lo, frac, jac = _warp_grid(dlnf)

    # fold the hann window (a function of the source row k) into the
    # interpolation weights so no extra pass over G is needed
    n = np.arange(K, dtype=np.float64)
    hann = (0.5 - 0.5 * np.cos(2.0 * np.pi * n / K)).astype(f32)
    lo = idx_lo.ravel()
    w_lo = ((f32(1.0) - frac) * jac).ravel() * hann[lo]
    w_hi = (frac * jac).ravel() * hann[lo + 1]

    # A[d, k, t]: sparse scatter (indices are unique -- lo vs lo+1 never
    # collide for the same t, and t differs otherwise)
    A = np.zeros((D, K, K_TAU), f32)
    d_idx = np.repeat(np.arange(D), K_TAU)
    t_idx = np.tile(np.arange(K_TAU), D)
    A[d_idx, lo, t_idx] = w_lo
    A[d_idx, lo + 1, t_idx] = w_hi

    try:
        from scipy.fft import rfft as _rfft

        W = _rfft(A, axis=-1, workers=-1)  # (D, K, FK) complex64
    except ImportError:
        W = np.fft.rfft(A, axis=-1)

    G = np.empty((D, K, 2048), f32)
    G[:, :, :FK] = W.real
    G[:, :, FK:] = W.imag[:, :, 1:1024]
    return G


def _build_frames32(x: np.ndarray) -> np.ndarray:
    """(B, N) f32 -> (K, ROWS_PAD) f32 transposed overlapped frames.

    ft[k, b*512 + w] = x[b, 512*w + k]  (w < 511; w = 511 zero)
    """
    ft = np.zeros((K, ROWS_PAD), np.float32)
    for b in range(B):
        frames = np.lib.stride_tricks.as_strided(
            x[b], shape=(NWIN, K), strides=(HOP * 4, 4)
        )
        ft[:, b * WPAD : b * WPAD + NWIN] = frames.T
    return ft


def _split8(a32: np.ndarray):
    """f32 -> (main fp8, residual fp8) with residual = q8(a - f32(main))."""
    a8 = a32.astype(F8)
    r8 = (a32 - a8.astype(np.float32)).astype(F8)
    return a8, r8


def _to_pairs(a: np.ndarray) -> np.ndarray:
    """(..., K, X) -> (..., NJ, 128, 2, X): k = 128*(2j + i) + p -> [j, p, i]."""
    lead = a.shape[:-2]
    x = a.shape[-1]
    a = a.reshape(*lead, NJ, 2, 128, x)
    order = tuple(range(len(lead))) + (
        len(lead),
        len(lead) + 2,
        len(lead) + 1,
        len(lead) + 3,
    )
    return np.ascontiguousarray(np.transpose(a, order))


def _prep_inputs(x: np.ndarray, dlnf: np.ndarray) -> dict:
    """Host prep: full-shape (pre-shard) device input arrays by name."""
    ft32 = _build_frames32(np.ascontiguousarray(np.asarray(x, np.float32)))
    g32 = _build_g32(np.asarray(dlnf, np.float32))
    ft8, ftr8 = _split8(ft32)
    g8, gr8 = _split8(g32)
    return {
        "ft8": _to_pairs(ft8),  # (4, 128, 2, ROWS_PAD) replicated
        "ftr8": _to_pairs(ftr8),  # (4, 128, 2, ROWS_PAD) replicated
        "g8": _to_pairs(g8),  # (D, 4, 128, 2, 2048) sharded on D
        "gr8": _to_pairs(gr8),  # (D, 4, 128, 2, 2048) sharded on D
    }


_REPLICATED = {"ft8", "ftr8"}


def _get_nc():
    if "nc" in _NC_CACHE:
        return _NC_CACHE["nc"]
    import concourse.bacc as bacc
    import concourse.mybir as mybir
    from concourse import tile

    nc = bacc.Bacc("TRN2", target_bir_lowering=False, debug=False, num_devices=NCORES)
    f8 = mybir.dt.float8e4
    ft8_d = nc.dram_tensor("ft8", [NJ, 128, 2, ROWS_PAD], f8, kind="ExternalInput")
    ftr8_d = nc.dram_tensor("ftr8", [NJ, 128, 2, ROWS_PAD], f8, kind="ExternalInput")
    g8_d = nc.dram_tensor("g8", [DPC, NJ, 128, 2, 2048], f8, kind="ExternalInput")
    gr8_d = nc.dram_tensor("gr8", [DPC, NJ, 128, 2, 2048], f8, kind="ExternalInput")
    out_d = nc.dram_tensor(
        "out", [DPC, ROWS_PAD, 2048], mybir.dt.float16, kind="ExternalOutput"
    )
    DR = mybir.MatmulPerfMode.DoubleRow

    with tile.TileContext(nc) as tc:
        with (
            tc.tile_pool(name="ftp", bufs=12) as ftp,
            tc.tile_pool(name="gp", bufs=24) as gp,
            tc.tile_pool(name="op", bufs=12) as op,
            tc.tile_pool(name="pp", bufs=8, space="PSUM") as pp,
        ):
            # DMA granularity is chosen so the first PSUM group's 16-tile
            # working set closes as early as possible: G comes in halves
            # (790ns each), frames in row-quads (500ns floor each), issued
            # interleaved in first-use order.
            def dma_g_j(d, j, g_t, gr_t, which=("g", "gr"), halves=(0, 1)):
                for name, dram, lst in (
                    ("g", g8_d, g_t),
                    ("gr", gr8_d, gr_t),
                ):
                    if name not in which:
                        continue
                    for h in halves:
                        t = gp.tile(
                            [128, 2, 1024], f8, name=f"{name}{d}_{j}_{h}", tag=name
                        )
                        nc.sync.dma_start(
                            t[:], dram[d, j, :, :, 1024 * h : 1024 * (h + 1)]
                        )
                        lst[j][h] = t

            # frames: halves of 1024 rows -> ft_t[j][rh], ftr_t[j][rh]
            # (790ns transfers match the ~650ns DMA issue cadence; smaller
            # tiles waste issue slots, bigger ones serialize transfers)
            ft_t = [[None] * 2 for _ in range(NJ)]
            ftr_t = [[None] * 2 for _ in range(NJ)]

            def dma_ft(j, rh):
                t = ftp.tile([128, 2, 1024], f8, name=f"ft{j}_{rh}", tag="ft")
                nc.sync.dma_start(t[:], ft8_d[j, :, :, 1024 * rh : 1024 * (rh + 1)])
                ft_t[j][rh] = t
                if j in INNER_J:
                    t = ftp.tile([128, 2, 1024], f8, name=f"ftr{j}_{rh}", tag="ftr")
                    nc.sync.dma_start(
                        t[:], ftr8_d[j, :, :, 1024 * rh : 1024 * (rh + 1)]
                    )
                    ftr_t[j][rh] = t

            # per-j issue order matches per-j consumption order of the first
            # PSUM half-group (h=0: g h0, ft q0, ftr q0, gr h0); h1 follows
            # issue ALL input DMAs upfront (SP queue), in first-use order;
            # output DMAs go on the Activation HWDGE queue so they cannot
            # head-of-line block the input feed
            gs = [
                [[None, None] for _ in range(NJ)],
                [[None, None] for _ in range(NJ)],
            ]
            grs = [
                [[None, None] for _ in range(NJ)],
                [[None, None] for _ in range(NJ)],
            ]
            for j in range(NJ):
                dma_g_j(0, j, gs[0], grs[0], which=("g",), halves=(0,))
                dma_ft(j, 0)
                if j in INNER_J:
                    dma_g_j(0, j, gs[0], grs[0], which=("gr",), halves=(0,))
            for j in range(NJ):
                dma_ft(j, 1)
            for j in range(NJ):
                dma_g_j(
                    0, j, gs[0], grs[0],
                    which=("g", "gr") if j in INNER_J else ("g",),
                    halves=(1,),
                )
            for d in range(1, DPC):
                for h in range(2):
                    for j in range(NJ):
                        dma_g_j(
                            d, j, gs[d], grs[d],
                            which=("g", "gr") if j in INNER_J else ("g",),
                            halves=(h,),
                        )

            # edge k-chunk pairs (j=0,3) carry ~8% of the output energy
            # (hann-weighted rows), so they get only the main fp8 term;
            # inner pairs get the full 3-way residual split
            steps = []
            for j in range(NJ):
                nsi = 3 if j in INNER_J else 1
                steps.extend((j, si) for si in range(nsi))

            def alloc_group(d, m, h):
                ost = op.tile(
                    [128, 1024], mybir.dt.float16, name=f"o{d}_{m}_{h}", tag="o"
                )
                pss = [
                    pp.tile(
                        [128, 512],
                        mybir.dt.float32,
                        name=f"p{d}_{m}_{h}_{nn}",
                        tag="p",
                    )
                    for nn in range(2)
                ]
                return ost, pss

            def emit_step(pss, g_t, gr_t, m, h, stepi):
                rh, lm = divmod(m, 8)
                j, si = steps[stepi]
                lt = ftr_t[j][rh] if si == 1 else ft_t[j][rh]
                rt = gr_t[j][h] if si == 2 else g_t[j][h]
                for nn in range(2):
                    nc.tensor.matmul(
                        pss[nn][:],
                        lt[:, :, 128 * lm : 128 * (lm + 1)],
                        rt[:, :, 512 * nn : 512 * (nn + 1)],
                        start=(stepi == 0),
                        stop=(stepi == len(steps) - 1),
                        perf_mode=DR,
                    )

            def emit_evict(ost, pss, d, m, h, gi):
                # one eviction copy on DVE, one on Act (total copy work is
                # 113us — more than the PE's 109us — so a single engine
                # would pace the kernel); out-DMAs all issue on SP, which is
                # idle once the upfront input issues drain (the large op
                # pool buffers the early output backlog)
                nc.vector.tensor_copy(ost[:, 0:512], pss[0][:])
                nc.scalar.copy(ost[:, 512:1024], pss[1][:])
                nc.sync.dma_start(
                    out_d[d, 128 * m : 128 * (m + 1), 1024 * h : 1024 * (h + 1)],
                    ost[:],
                )

            for d in range(DPC):
                g_t, gr_t = gs[d], grs[d]
                for h in range(2):
                    if d == 0 and h == 0:
                        # startup is DMA-feed-bound: software-pipeline the
                        # first four m-groups diagonally so PE consumption
                        # matches tile arrival AND the groups free their
                        # PSUM banks staggered (not all at once)
                        NB = 4
                        ns = len(steps)
                        batch = [alloc_group(0, m, 0) for m in range(NB)]
                        for w in range(ns + NB - 1):
                            for m in range(NB):
                                s = w - m
                                if 0 <= s < ns:
                                    emit_step(batch[m][1], g_t, gr_t, m, 0, s)
                            if w >= ns - 1:
                                m = w - (ns - 1)
                                emit_evict(batch[m][0], batch[m][1], 0, m, 0, m)
                        rest = range(NB, 16)
                    else:
                        rest = range(16)
                    for m in rest:
                        gi = (2 * d + h) * 16 + m  # global half-group index
                        ost, pss = alloc_group(d, m, h)
                        for stepi in range(len(steps)):
                            emit_step(pss, g_t, gr_t, m, h, stepi)
                        emit_evict(ost, pss, d, m, h, gi)
    nc.compile()
    _NC_CACHE["nc"] = nc
    return nc


def _get_runner():
    """Build (once) a sharded jitted callable over the 8 cores.

    Mirrors the multi-core tail of bass2jax.run_bass_via_pjrt, but caches
    the jitted function so repeat kernel() calls don't re-trace/re-compile.
    Returns (fn, in_names, out_names, out_shapes_dtypes, mesh).
    """
    if "runner" in _NC_CACHE:
        return _NC_CACHE["runner"]
    import jax
    import concourse.mybir as mybir
    from concourse import bass2jax
    from jax.sharding import Mesh, PartitionSpec
    from jax.experimental.shard_map import shard_map

    nc = _get_nc()
    bass2jax.install_neuronx_cc_hook()

    partition_name = (
        nc.partition_id_tensor.name if nc.partition_id_tensor is not None else None
    )
    in_names = []
    out_names = []
    out_avals = []
    for alloc in nc.m.functions[0].allocations:
        if not isinstance(alloc, mybir.MemoryLocationSet):
            continue
        name = alloc.memorylocations[0].name
        if alloc.kind == "ExternalInput":
            if name != partition_name:
                in_names.append(name)
        elif alloc.kind == "ExternalOutput":
            shape = tuple(alloc.tensor_shape)
            dtype = mybir.dt.np(alloc.dtype)
            out_names.append(name)
            out_avals.append(jax.core.ShapedArray(shape, dtype))
    n_params = len(in_names)
    n_outs = len(out_names)
    all_names = list(in_names) + list(out_names)
    if partition_name is not None:
        all_names.append(partition_name)
    all_names = tuple(all_names)

    def _body(*args):
        operands = list(args)
        if partition_name is not None:
            operands.append(bass2jax.partition_id_tensor())
        outs = bass2jax._bass_exec_p.bind(
            *operands,
            out_avals=tuple(out_avals),
            in_names=all_names,
            out_names=tuple(out_names),
            lowering_input_output_aliases=(),
            sim_require_finite=True,
            sim_require_nnan=True,
            nc=nc,
        )
        return tuple(outs)

    devices = jax.devices()[:NCORES]
    mesh = Mesh(np.asarray(devices), ("core",))
    # frames are identical on every core: pass them replicated (one wire
    # transfer); every other operand is sharded on axis 0.
    in_specs = tuple(
        PartitionSpec() if name in _REPLICATED else PartitionSpec("core")
        for name in in_names
    ) + (PartitionSpec("core"),) * n_outs
    out_specs = (PartitionSpec("core"),) * n_outs
    fn = jax.jit(
        shard_map(
            _body, mesh=mesh, in_specs=in_specs, out_specs=out_specs, check_rep=False
        ),
        donate_argnums=tuple(range(n_params, n_params + n_outs)),
        keep_unused=True,
    )
    runner = (fn, in_names, out_names, [(a.shape, a.dtype) for a in out_avals], mesh)
    _NC_CACHE["runner"] = runner
    return runner


def kernel(x: np.ndarray, dlnf: np.ndarray) -> np.ndarray:
    full_in = _prep_inputs(x, dlnf)

    fn, in_names, out_names, out_sd, _mesh = _get_runner()
    concat_in = [np.ascontiguousarray(full_in[name]) for name in in_names]
    concat_zeros = [np.zeros((NCORES * s[0], *s[1:]), dt) for (s, dt) in out_sd]
    out_arrs = fn(*concat_in, *concat_zeros)
    o_all = np.asarray(out_arrs[out_names.index("out")]).reshape(
        NCORES, DPC, ROWS_PAD, 2048
    )

    # o5[d, b, w, :] with w < NWIN valid; assemble into interleaved complex64
    # via a float32 view so the fp16->f32 cast fuses into the strided copies
    o5 = o_all.reshape(D, B, WPAD, 2048)[:, :, :NWIN, :]
    out_f = np.empty((B, NWIN, D, FK, 2), np.float32)
    out_f[..., 0] = np.transpose(o5[:, :, :, :FK], (1, 2, 0, 3))
    out_f[:, :, :, 0, 1] = 0.0
    out_f[:, :, :, 1024, 1] = 0.0
    out_f[:, :, :, 1:1024, 1] = np.transpose(o5[:, :, :, FK:], (1, 2, 0, 3))
    return out_f.view(np.complex64)[..., 0]
